# revision 65
# baseline (speedup 1.0000x reference)
"""TRN2 Bass kernel for nn_AttentionBlock (GroupNorm32 + 8-head attention + proj + residual).

Sharding: data-parallel over batch — batch=8, one batch element per NeuronCore, no
collectives.

Per core: GroupNorm stats per 128-channel chunk (sum and sum-of-squares fall out of
ACT Identity/Square activations via accum_out, grouped by tiny mask matmuls,
rsqrt as exp(-0.5*ln v) so one ACT table set serves the whole kernel); qkv, attention
and proj as bf16 matmuls on TensorE (score matmuls for a head pair run concurrently
in the two 64-row PE groups); softmax exp on ScalarE in one (128,1024) activation per
score block; the attention matmul uses vT with an appended ones-column so the softmax
denominator falls out of the same accumulation, and the division is 1/den =
exp(-ln den): the four 512-wide denominator rows of a head pair are folded to
(128,16) by tiny SBUF->SBUF DMAs so the ACT ln/exp is ~0.6us, unfolded back,
broadcast on GpSimd (final pair: tiny f32 PE matmuls into spare
PSUM banks, since the PE is briefly idle there) and multiplied on DVE.
Schedule-shaping for the in-order engines: PE warm-up matmuls cover the stats
startup, later head-pairs' q/k matmuls are deferred into earlier pairs' loops as PE
filler (the attention steady state is ACT-bound), divisions are software-pipelined
into the next pair's loop, and proj runs k-outer waves across all 8 PSUM banks.

Numerics: all matmuls bf16 with fp32 PSUM accumulation (end-to-end ~1.9e-4 rel-l2 vs
the fp32 reference); everything else fp32.

Self-contained: hardcodes shapes from the problem spec (x (8,512,32,32) f32 etc).
"""
import numpy as np
import ml_dtypes

B, CH, HH, WW = 8, 512, 32, 32
L = HH * WW                  # 1024
HEADS = 8
GROUPS = 32
EPS = 1e-5
DH = CH // HEADS             # 64
KC = CH // 128               # 4 c-chunks
OC3 = 3 * CH // 128          # 12 qkv o-chunks
SC = L // 128                # 8 s/l-chunks
TC = L // 512                # 2 t-chunks
GN_N = (CH // GROUPS) * L    # elements per group = 16384
DEFER_QK = True
SCOPES = False

_cache = {}


def _build(has_qkv_bias, has_proj_bias, debug=False):
    import concourse.bass as bass
    import concourse.tile as tile
    from concourse import bacc, mybir
    import bass_rust as _bass_rust
    from concourse.hw_specs import get_activation_tables

    F32 = mybir.dt.float32
    BF16 = mybir.dt.bfloat16
    AF = mybir.ActivationFunctionType
    OP = mybir.AluOpType
    AX = mybir.AxisListType

    class _Bacc(bacc.Bacc):
        # Pin Exp/Ln to the combined `natural_log_exp_and_others` table set so
        # alternating Ln/Exp activations don't thrash ACT_TABLE_LOADs (~2.7us
        # each). Same algorithm as Bacc.insert_act_table_loads, with Exp/Ln
        # stripped from every other set so the chooser can't pick them.
        def insert_act_table_loads(self):
            has_activation = any(
                isinstance(i, mybir.InstActivation)
                for b in self.main_func.blocks
                for i in b.instructions
            )
            if not has_activation:
                return
            combo = {AF.Exp, AF.Ln}
            tables = []
            for name, fns in get_activation_tables(self.m.arch).items():
                if name != "natural_log_exp_and_others":
                    fns = {f for f in fns if f not in combo}
                tables.append((name, fns))
            _bass_rust.insert_act_table_loads(self, tables)

    nc = _Bacc("TRN2", target_bir_lowering=False, debug=False, num_devices=8)

    x_d = nc.dram_tensor("x", [CH, L], F32, kind="ExternalInput").ap()
    qw_d = nc.dram_tensor("qkv_wt", [CH, 3 * CH], BF16, kind="ExternalInput").ap()
    pw_d = nc.dram_tensor("proj_wt", [CH, CH], BF16, kind="ExternalInput").ap()
    gmask_d = nc.dram_tensor("gmask", [128, 8], F32, kind="ExternalInput").ap()
    gmaskT_d = nc.dram_tensor("gmask_t", [8, 128], F32, kind="ExternalInput").ap()
    if has_qkv_bias:
        qkb_d = nc.dram_tensor("qk_bias", [128, 8], F32, kind="ExternalInput").ap()
        vb_d = nc.dram_tensor("v_bias", [128, KC], F32, kind="ExternalInput").ap()
    if has_proj_bias:
        pb_d = nc.dram_tensor("p_bias", [128, KC], F32, kind="ExternalInput").ap()
    out_d = nc.dram_tensor("out", [CH, L], F32, kind="ExternalOutput").ap()
    if debug:
        dbg = {
            "d_xhat": nc.dram_tensor("d_xhat", [128, KC * L], F32, kind="ExternalOutput").ap(),
            "d_qk": nc.dram_tensor("d_qk", [128, 8 * L], F32, kind="ExternalOutput").ap(),
            "d_vt": nc.dram_tensor("d_vt", [128, SC * HEADS * 65], F32, kind="ExternalOutput").ap(),
            "d_asb": nc.dram_tensor("d_asb", [128, KC * L], F32, kind="ExternalOutput").ap(),
            "d_ew0": nc.dram_tensor("d_ew0", [128, L], F32, kind="ExternalOutput").ap(),
        }

    with tile.TileContext(nc) as tc:
        import contextlib
        ctx = contextlib.ExitStack()
        pers = ctx.enter_context(tc.tile_pool(name="pers", bufs=1))
        scr = ctx.enter_context(tc.tile_pool(name="scr", bufs=2))
        ewp = ctx.enter_context(tc.tile_pool(name="ewp", bufs=8))
        dvp = ctx.enter_context(tc.tile_pool(name="dvp", bufs=2))
        asg = ctx.enter_context(tc.tile_pool(name="asg", bufs=8))
        outp = ctx.enter_context(tc.tile_pool(name="outp", bufs=3))

        # ---- PE warmup: keep HAM at K=8/8 through the stats/DMA startup chain ----
        with tc.tile_pool(name="psW", bufs=1, space="PSUM") as psW:
            wsrc = scr.tile([128, 640], BF16, tag="wsrc")
            nc.gpsimd.memset(wsrc[:], 0.0)
            wps = psW.tile([128, 512], F32, tag="warm")
            for _ in range(64):
                nc.tensor.matmul(wps[:], wsrc[:, 0:128], wsrc[:, 128:640],
                                 start=True, stop=True)

        # ---- load inputs ----
        xs = pers.tile([128, KC * L], F32, tag="xs")
        for k in range(KC):
            nc.sync.dma_start(xs[:, k * L:(k + 1) * L], x_d[128 * k:128 * (k + 1), :])
        qw = pers.tile([128, KC * 3 * CH], BF16, tag="qw")
        for k in range(KC):
            nc.sync.dma_start(qw[:, k * 3 * CH:(k + 1) * 3 * CH],
                              qw_d[128 * k:128 * (k + 1), :])
        pw = pers.tile([128, KC * CH], BF16, tag="pw")
        for k in range(KC):
            nc.sync.dma_start(pw[:, k * CH:(k + 1) * CH], pw_d[128 * k:128 * (k + 1), :])
        gmask = pers.tile([128, 8], F32, tag="gmask")
        nc.sync.dma_start(gmask[:], gmask_d[:])
        gmaskT = pers.tile([8, 128], F32, tag="gmask_t")
        nc.sync.dma_start(gmaskT[:], gmaskT_d[:])
        if has_qkv_bias:
            qkb = pers.tile([128, 8], F32, tag="qkb")
            nc.sync.dma_start(qkb[:], qkb_d[:])
            vb = pers.tile([128, KC], F32, tag="vb")
            nc.sync.dma_start(vb[:], vb_d[:])
        if has_proj_bias:
            pb = pers.tile([128, KC], F32, tag="pb")
            nc.sync.dma_start(pb[:], pb_d[:])

        # ---- GroupNorm statistics + xhat, fully per-chunk so qkv can start on
        # chunk 0 while chunk 3 is still being reduced ----
        qkv_psum = tc.tile_pool(name="psQ", bufs=8, space="PSUM")
        psQ = qkv_psum.__enter__()
        epsb = pers.tile([8, 1], F32, tag="epsb")
        nc.gpsimd.memset(epsb[:], EPS)
        ones64 = pers.tile([1, 64], F32, tag="ones64")
        nc.gpsimd.memset(ones64[:], 1.0)
        # trigger the (single) ACT table load off the critical path
        tldt = pers.tile([8, 1], F32, tag="tldt")
        nc.scalar.activation(tldt[:], epsb[:], AF.Exp)

        stat = pers.tile([128, 8], F32, tag="stat")  # cols 2k: sum(x), 2k+1: sum(x^2)
        xhat = pers.tile([128, KC * L], BF16, tag="xhat")
        bc = pers.tile([128, 2 * KC], F32, tag="bc")  # cols 2k mean, 2k+1 rstd
        for k in range(KC):
            xk = xs[:, k * L:(k + 1) * L]
            sq = scr.tile([128, L], F32, tag="sq")
            nc.scalar.activation(sq[:], xk, AF.Identity,
                                 accum_out=stat[:, 2 * k:2 * k + 1])
            sq2 = scr.tile([128, L], F32, tag="sq")
            nc.scalar.activation(sq2[:], xk, AF.Square,
                                 accum_out=stat[:, 2 * k + 1:2 * k + 2])
            gst_ps = psQ.tile([8, 2], F32, tag="ps")
            nc.tensor.matmul(gst_ps[:], gmask[:], stat[:, 2 * k:2 * k + 2],
                             start=True, stop=True)
            s2k = pers.tile([8, 2], F32, tag=f"s2k{k}")   # col 0 mean, col 1 rstd
            vk = pers.tile([8, 2], F32, tag=f"vk{k}")     # col 0 var, col 1 scratch
            nc.vector.tensor_scalar_mul(s2k[:], gst_ps[:], 1.0 / GN_N)  # mean, E[x^2]
            nc.vector.tensor_mul(vk[:, 1:2], s2k[:, 0:1], s2k[:, 0:1])  # mean^2
            nc.vector.tensor_sub(vk[:, 0:1], s2k[:, 1:2], vk[:, 1:2])   # var
            nc.scalar.activation(vk[:, 1:2], vk[:, 0:1], AF.Ln, bias=epsb[:])
            nc.scalar.activation(s2k[:, 1:2], vk[:, 1:2], AF.Exp, scale=-0.5)
            bc_ps = psQ.tile([128, 2], F32, tag="ps")
            nc.tensor.matmul(bc_ps[:], gmaskT[:], s2k[:], start=True, stop=True)
            nc.vector.tensor_copy(bc[:, 2 * k:2 * k + 2], bc_ps[:])
            nmr = pers.tile([128, 1], F32, tag=f"nmr{k}")   # -mean*rstd
            nc.vector.tensor_scalar(
                out=nmr[:], in0=bc[:, 2 * k:2 * k + 1],
                scalar1=bc[:, 2 * k + 1:2 * k + 2], scalar2=-1.0,
                op0=OP.mult, op1=OP.mult)
            nc.scalar.activation(xhat[:, k * L:(k + 1) * L], xk, AF.Identity,
                                 bias=nmr[:], scale=bc[:, 2 * k + 1:2 * k + 2])
            # q/k wave k for heads 0-2 (6 matmuls needing only xhat chunk k),
            # emitted here so the in-order PE fills the next chunk's stats wait.
            # 6 held tiles + 2 rotating stats tiles fit the 8 PSUM banks.
            if k == 0:
                up_tiles = {}
                for j_ in (0, 4, 1):
                    for t_ in range(TC):
                        ps = psQ.tile([128, 512], F32, tag="ps")
                        up_tiles[(j_, t_)] = ps
            for j_ in (0, 4, 1):
                for t_ in range(TC):
                    nc.tensor.matmul(
                        up_tiles[(j_, t_)][:],
                        qw[:, k * 3 * CH + 128 * j_:k * 3 * CH + 128 * (j_ + 1)],
                        xhat[:, k * L + 512 * t_:k * L + 512 * (t_ + 1)],
                        start=(k == 0), stop=(k == KC - 1))

        if debug:
            def dump_bf16(dram_ap, sb_ap, width):
                for off in range(0, width, 512):
                    w = min(512, width - off)
                    stg = outp.tile([128, 512], F32, tag="dstg")
                    nc.vector.tensor_copy(stg[:sb_ap.shape[0], :w],
                                          sb_ap[:, off:off + w])
                    nc.sync.dma_start(dram_ap[:sb_ap.shape[0], off:off + w],
                                      stg[:sb_ap.shape[0], :w])
            dump_bf16(dbg["d_xhat"], xhat[:], KC * L)

        # ---- qkv: q,k in (o, l) layout; v transposed to (l, vc) with ones column.
        # Only pairs 0-1's q/k (j=0,4,1,5) and vT are computed up front; the
        # rest are emitted inside the attention pair loops as PE filler (the
        # attention phase is ACT-bound, in-order PE needs real work in-stream).
        qk = pers.tile([128, 8 * L], BF16, tag="qk")   # o-chunk j: cols j*L..; j=0-3 q, 4-7 k

        def emit_qk(j, pool, width):
            for t in range(TC):
                ps = pool.tile([128, width], F32, tag="ps")
                for k in range(KC):
                    nc.tensor.matmul(
                        ps[:, 0:512],
                        qw[:, k * 3 * CH + 128 * j:k * 3 * CH + 128 * (j + 1)],
                        xhat[:, k * L + 512 * t:k * L + 512 * (t + 1)],
                        start=(k == 0), stop=(k == KC - 1))
                dst = qk[:, j * L + 512 * t:j * L + 512 * (t + 1)]
                if has_qkv_bias:
                    nc.vector.tensor_scalar_add(dst, ps[:, 0:512], qkb[:, j:j + 1])
                else:
                    nc.vector.tensor_copy(dst, ps[:, 0:512])

        import contextlib as _ctxlib
        def _scope(name):
            return tc.spectator_scope(name) if SCOPES else _ctxlib.nullcontext()
        with _scope("qkv"):
            for j_ in (0, 4, 1):
                for t_ in range(TC):
                    dst = qk[:, j_ * L + 512 * t_:j_ * L + 512 * (t_ + 1)]
                    if has_qkv_bias:
                        nc.vector.tensor_scalar_add(dst, up_tiles[(j_, t_)][:],
                                                    qkb[:, j_:j_ + 1])
                    else:
                        nc.vector.tensor_copy(dst, up_tiles[(j_, t_)][:])
            for j in ((5,) if DEFER_QK else (5, 2, 6, 3, 7)):
                emit_qk(j, psQ, 512)
            vt = pers.tile([128, SC * (HEADS * 65)], BF16, tag="vt")
            for lc in range(SC):
                v3 = vt[:, lc * 520:(lc + 1) * 520].rearrange("p (h c) -> p h c", c=65)
                nc.gpsimd.memset(v3[:, :, 64:65], 1.0)
            for lc in range(SC):
                ps = psQ.tile([128, 512], F32, tag="ps")
                for k in range(KC):
                    nc.tensor.matmul(
                        ps[:], xhat[:, k * L + 128 * lc:k * L + 128 * (lc + 1)],
                        qw[:, k * 3 * CH + 2 * CH:k * 3 * CH + 3 * CH],
                        start=(k == 0), stop=(k == KC - 1))
                v3 = vt[:, lc * 520:(lc + 1) * 520].rearrange("p (h c) -> p h c", c=65)
                src = ps[:].rearrange("p (h c) -> p h c", c=64)
                nc.vector.tensor_copy(v3[:, :, 0:64], src)
        qkv_psum.__exit__(None, None, None)

        if debug:
            dump_bf16(dbg["d_qk"], qk[:], 8 * L)
            dump_bf16(dbg["d_vt"], vt[:], SC * HEADS * 65)

        # ---- attention, head pairs (2m, 2m+1) packed into PE row groups ----
        a_sb = pers.tile([128, KC * L], BF16, tag="a_sb")
        attn_psum = tc.tile_pool(name="psS", bufs=2, space="PSUM")
        psS = attn_psum.__enter__()
        attn_acc = tc.tile_pool(name="psA", bufs=4, space="PSUM")
        psA = attn_acc.__enter__()

        def div_recip(stgs):
            # Fold the four 512-wide ones-row sums into (128,16) via tiny
            # SBUF->SBUF DMAs (DMA engines are idle here) so the ACT ln/exp
            # for 1/den costs ~0.6us instead of ~4us of 1-partition work,
            # then unfold back to a partition-0 row for the gpsimd broadcast.
            den128 = dvp.tile([128, 16], F32, tag="d128")
            for i, (sg, e, t, mm_) in enumerate(stgs):
                nc.sync.dma_start(den128[:, 4 * i:4 * (i + 1)], sg[64:65, :])
            ln128 = dvp.tile([128, 16], F32, tag="l128")
            nc.scalar.activation(ln128[:], den128[:], AF.Ln)
            r128 = dvp.tile([128, 16], F32, tag="r128")
            nc.scalar.activation(r128[:], ln128[:], AF.Exp, scale=-1.0)
            rden = dvp.tile([1, 4 * 512], F32, tag="rden")
            for i in range(4):
                nc.sync.dma_start(rden[0:1, 512 * i:512 * (i + 1)],
                                  r128[:, 4 * i:4 * (i + 1)])
            return rden

        def div_mul(rden, i, sg, e, t, mm_):
            bsb = dvp.tile([64, 512], F32, tag="bsb")
            nc.gpsimd.partition_broadcast(bsb[:], rden[0:1, 512 * i:512 * (i + 1)])
            dst = a_sb[64 * e:64 * (e + 1),
                       mm_ * L + 512 * t:mm_ * L + 512 * (t + 1)]
            nc.vector.tensor_mul(dst, sg[0:64, :], bsb[:])
            if has_qkv_bias:
                nc.vector.tensor_scalar_add(
                    dst, dst, vb[64 * e:64 * (e + 1), mm_:mm_ + 1])

        def division_steps(stgs):
            # generator: one cheap step per scheduling slot
            rden = div_recip(stgs)
            yield
            for i, (sg, e, t, mm_) in enumerate(stgs):
                div_mul(rden, i, sg, e, t, mm_)
                if i % 2 == 1:
                    yield

        pending_div = None
        for m in range(4):
            with _scope(f"attn{m}"):
                ps_a = [[None, None], [None, None]]
                for e in range(2):
                    for t in range(TC):
                        pa = psA.tile([65, 512], F32, tag="pa")
                        ps_a[e][t] = pa

                def q_ap(e, t):
                    return qk[64 * e:64 * (e + 1), m * L + 512 * t:m * L + 512 * (t + 1)]

                def k_ap(e, sc):
                    return qk[64 * e:64 * (e + 1),
                              (4 + m) * L + 128 * sc:(4 + m) * L + 128 * (sc + 1)]

                def attn_mm(sc, e):
                    ew = ew_tiles[(sc, e)]
                    for t in range(TC):
                        nc.tensor.matmul(
                            ps_a[e][t][:],
                            vt[:, sc * 520 + (2 * m + e) * 65:
                               sc * 520 + (2 * m + e) * 65 + 65],
                            ew[:, 512 * t:512 * (t + 1)],
                            start=(sc == 0), stop=(sc == SC - 1))

                ew_tiles = {}
                for sc in range(SC):
                    ps_w = [None, None]
                    for e in range(2):
                        pw_t = psS.tile([128, 1024], F32, tag="ps")
                        ps_w[e] = pw_t
                    # packed score MM pairs (head 2m rows 0-63, head 2m+1 rows 64-127)
                    for t in range(TC):
                        for e in range(2):
                            nc.tensor.matmul(ps_w[e][:, 512 * t:512 * (t + 1)],
                                             k_ap(e, sc), q_ap(e, t),
                                             start=True, stop=True)
                    for e in range(2):
                        ew = ewp.tile([128, L], BF16, tag="ew")
                        ew_tiles[(sc, e)] = ew
                        nc.scalar.activation(ew[:], ps_w[e][:], AF.Exp)
                    if debug and m == 0 and sc == 0:
                        dump_bf16(dbg["d_ew0"], ew_tiles[(0, 0)][:], L)
                    # previous pair's division, one step per sc to spread the load
                    if pending_div is not None:
                        next(pending_div, None)
                    # deferred q/k matmuls for pair m+2 act as PE filler in the
                    # ACT-bound attention steady state
                    if DEFER_QK:
                        if m < 2 and sc == 2:
                            emit_qk(m + 2, psS, 1024)
                        if m < 2 and sc == 5:
                            emit_qk(4 + m + 2, psS, 1024)
                    # software-pipeline: attn MMs for sc-1 after scores for sc
                    if sc > 0:
                        for e in range(2):
                            attn_mm(sc - 1, e)
                for e in range(2):
                    attn_mm(SC - 1, e)

                # stage accumulators to SBUF so the PSUM banks free up for the
                # next head pair; the divisions run interleaved with the NEXT
                # pair's exp stream (pending_div) to avoid an ACT lump here.
                if pending_div is not None:
                    for _ in pending_div:  # flush any leftovers of pair m-1
                        pass
                stgs = []
                for e in range(2):
                    for t in range(TC):
                        sg = asg.tile([65, 512], F32, tag="astg")
                        nc.vector.tensor_copy(sg[:], ps_a[e][t][:])
                        stgs.append((sg, e, t, m))
                if m < 3:
                    pending_div = division_steps(stgs)
                else:
                    pending_div = None
                    final_stgs = stgs
        attn_acc.__exit__(None, None, None)
        attn_psum.__exit__(None, None, None)

        if debug:
            dump_bf16(dbg["d_asb"], a_sb[:], KC * L)

        # ---- proj + residual: k-outer waves across all 8 PSUM banks, so the
        # first 24 matmuls only need a_sb chunks 0-2 and overlap the final
        # division flush ----
        with tc.tile_pool(name="psP", bufs=6, space="PSUM") as psP, \
             tc.tile_pool(name="psB", bufs=2, space="PSUM") as psB, \
             _scope("proj"):
            for t in range(TC):
                pstiles = {}
                for i in range(KC):
                    ps = psP.tile([128, 512], F32, tag="ps")
                    pstiles[i] = ps
                for k in range(KC):
                    if t == 0 and k == 1:
                        # final pair's reciprocal: DMA-fold + ACT, off the PE stream
                        final_rden = div_recip(final_stgs)
                    if t == 0 and k == 3:
                        # broadcast on the (briefly idle) PE into spare banks
                        for i_, (sg, e, tt, mm_) in enumerate(final_stgs):
                            pb_ps = psB.tile([64, 512], F32, tag="pb")
                            nc.tensor.matmul(
                                pb_ps[:], ones64[:],
                                final_rden[0:1, 512 * i_:512 * (i_ + 1)],
                                start=True, stop=True)
                            dst = a_sb[64 * e:64 * (e + 1),
                                       mm_ * L + 512 * tt:mm_ * L + 512 * (tt + 1)]
                            nc.vector.tensor_mul(dst, sg[0:64, :], pb_ps[:])
                            if has_qkv_bias:
                                nc.vector.tensor_scalar_add(
                                    dst, dst, vb[64 * e:64 * (e + 1), mm_:mm_ + 1])
                    for i in range(KC):
                        nc.tensor.matmul(
                            pstiles[i][:],
                            pw[:, k * CH + 128 * i:k * CH + 128 * (i + 1)],
                            a_sb[:, k * L + 512 * t:k * L + 512 * (t + 1)],
                            start=(k == 0), stop=(k == KC - 1))
                for i in range(KC):
                    ot = outp.tile([128, 512], F32, tag="ot")
                    nc.vector.tensor_add(ot[:],
                                         xs[:, i * L + 512 * t:i * L + 512 * (t + 1)],
                                         pstiles[i][:])
                    if has_proj_bias:
                        nc.vector.tensor_scalar_add(ot[:], ot[:], pb[:, i:i + 1])
                    nc.sync.dma_start(
                        out_d[128 * i:128 * (i + 1), 512 * t:512 * (t + 1)], ot[:])
        ctx.close()

    nc.compile()
    return nc


def _prep_inputs(x, norm_w, norm_b, qkv_w, qkv_b, proj_w, proj_b):
    scale = DH ** -0.25
    w_eff = (qkv_w.astype(np.float64) * norm_w.astype(np.float64)[None, :])
    b_eff = qkv_b.astype(np.float64) + w_eff @ norm_b.astype(np.float64)
    # reference splits qkv per head: row h*192 + {0:64 q, 64:128 k, 128:192 v}.
    # device layout wants [q_all_heads | k_all_heads | v_all_heads], head-major.
    perm = np.concatenate([
        np.concatenate([np.arange(h * 3 * DH + t * DH, h * 3 * DH + (t + 1) * DH)
                        for h in range(HEADS)])
        for t in range(3)])
    w_eff = w_eff[perm]
    b_eff = b_eff[perm]
    w_eff[:2 * CH] *= scale
    b_eff[:2 * CH] *= scale
    qkv_wt = np.ascontiguousarray(w_eff.T).astype(np.float32).astype(ml_dtypes.bfloat16)
    proj_wt = np.ascontiguousarray(proj_w.T).astype(ml_dtypes.bfloat16)

    p = np.arange(128)
    gmask = (p[:, None] // 16 == np.arange(8)[None, :]).astype(np.float32)
    gmask_t = np.ascontiguousarray(gmask.T)

    has_qkv_bias = bool(np.any(b_eff != 0.0))
    has_proj_bias = bool(np.any(proj_b != 0.0))
    common = {"qkv_wt": qkv_wt, "proj_wt": proj_wt, "gmask": gmask,
              "gmask_t": gmask_t}
    if has_qkv_bias:
        qk_part = b_eff[:2 * CH].astype(np.float32).reshape(8, 128).T
        v_part = b_eff[2 * CH:].astype(np.float32).reshape(KC, 128).T
        common["qk_bias"] = np.ascontiguousarray(qk_part)
        common["v_bias"] = np.ascontiguousarray(v_part)
    if has_proj_bias:
        common["p_bias"] = np.ascontiguousarray(
            proj_b.astype(np.float32).reshape(KC, 128).T)
    xf = np.ascontiguousarray(x.reshape(B, CH, L)).astype(np.float32)
    in_maps = [dict(common, x=np.ascontiguousarray(xf[i])) for i in range(B)]
    return in_maps, has_qkv_bias, has_proj_bias


def _get_nc(flags):
    if flags not in _cache:
        _cache[flags] = _build(*flags)
    return _cache[flags]


def _run(inputs, trace=False, tmpdir=None):
    import time
    from concourse.bass_utils import run_bass_kernel_spmd
    in_maps, hqb, hpb = _prep_inputs(**inputs)
    nc = _get_nc((hqb, hpb))
    kw = {}
    if trace:
        kw = dict(trace=True, tmpdir=tmpdir)
    last_err = None
    for attempt in range(3):
        # the very first execution on a freshly-attached device occasionally
        # fails with NRT_EXEC_UNIT_UNRECOVERABLE; a retry recovers it
        try:
            res = run_bass_kernel_spmd(nc, in_maps, list(range(B)), **kw)
            break
        except Exception as e:  # noqa: BLE001
            last_err = e
            time.sleep(5)
    else:
        raise last_err
    out = np.stack([res.results[i]["out"] for i in range(B)])
    return out.reshape(B, CH, HH, WW).astype(np.float32), res


def kernel(x, norm_w, norm_b, qkv_w, qkv_b, proj_w, proj_b):
    out, _ = _run(dict(x=x, norm_w=norm_w, norm_b=norm_b, qkv_w=qkv_w,
                       qkv_b=qkv_b, proj_w=proj_w, proj_b=proj_b))
    return out


# revision 67
# speedup vs baseline: 1.0313x; 1.0313x over previous
"""TRN2 Bass kernel for nn_AttentionBlock (GroupNorm32 + 8-head attention + proj + residual).

Sharding: data-parallel over batch — batch=8, one batch element per NeuronCore, no
collectives.

Per core: GroupNorm stats per 128-channel chunk (sum and sum-of-squares fall out of
ACT Identity/Square activations via accum_out, grouped by tiny mask matmuls,
rsqrt as exp(-0.5*ln v) so one ACT table set serves the whole kernel); qkv, attention
and proj as bf16 matmuls on TensorE (score matmuls for a head pair run concurrently
in the two 64-row PE groups); softmax exp on ScalarE in one (128,1024) activation per
score block; the attention matmul uses vT with an appended ones-column so the softmax
denominator falls out of the same accumulation, and the division is 1/den =
exp(-ln den): the four 512-wide denominator rows of a head pair are folded to
(128,16) by tiny SBUF->SBUF DMAs so the ACT ln/exp is ~0.6us, unfolded back,
broadcast on GpSimd (final pair: tiny f32 PE matmuls into spare
PSUM banks, since the PE is briefly idle there) and multiplied on DVE.
Schedule-shaping for the in-order engines: PE warm-up matmuls cover the stats
startup, later head-pairs' q/k matmuls are deferred into earlier pairs' loops as PE
filler (the attention steady state is ACT-bound), divisions are software-pipelined
into the next pair's loop, and proj runs k-outer waves across all 8 PSUM banks.

Numerics: all matmuls bf16 with fp32 PSUM accumulation (end-to-end ~1.9e-4 rel-l2 vs
the fp32 reference); everything else fp32.

Self-contained: hardcodes shapes from the problem spec (x (8,512,32,32) f32 etc).
"""
import numpy as np
import ml_dtypes

B, CH, HH, WW = 8, 512, 32, 32
L = HH * WW                  # 1024
HEADS = 8
GROUPS = 32
EPS = 1e-5
DH = CH // HEADS             # 64
KC = CH // 128               # 4 c-chunks
OC3 = 3 * CH // 128          # 12 qkv o-chunks
SC = L // 128                # 8 s/l-chunks
TC = L // 512                # 2 t-chunks
GN_N = (CH // GROUPS) * L    # elements per group = 16384
DEFER_QK = True
SCOPES = False

_cache = {}


def _build(has_qkv_bias, has_proj_bias, debug=False):
    import concourse.bass as bass
    import concourse.tile as tile
    from concourse import bacc, mybir
    import bass_rust as _bass_rust
    from concourse.hw_specs import get_activation_tables

    F32 = mybir.dt.float32
    BF16 = mybir.dt.bfloat16
    AF = mybir.ActivationFunctionType
    OP = mybir.AluOpType
    AX = mybir.AxisListType

    class _Bacc(bacc.Bacc):
        # Pin Exp/Ln to the combined `natural_log_exp_and_others` table set so
        # alternating Ln/Exp activations don't thrash ACT_TABLE_LOADs (~2.7us
        # each). Same algorithm as Bacc.insert_act_table_loads, with Exp/Ln
        # stripped from every other set so the chooser can't pick them.
        def insert_act_table_loads(self):
            has_activation = any(
                isinstance(i, mybir.InstActivation)
                for b in self.main_func.blocks
                for i in b.instructions
            )
            if not has_activation:
                return
            combo = {AF.Exp, AF.Ln}
            tables = []
            for name, fns in get_activation_tables(self.m.arch).items():
                if name != "natural_log_exp_and_others":
                    fns = {f for f in fns if f not in combo}
                tables.append((name, fns))
            _bass_rust.insert_act_table_loads(self, tables)

    nc = _Bacc("TRN2", target_bir_lowering=False, debug=False, num_devices=8)

    x_d = nc.dram_tensor("x", [CH, L], F32, kind="ExternalInput").ap()
    qw_d = nc.dram_tensor("qkv_wt", [CH, 3 * CH], BF16, kind="ExternalInput").ap()
    pw_d = nc.dram_tensor("proj_wt", [CH, CH], BF16, kind="ExternalInput").ap()
    gmask_d = nc.dram_tensor("gmask", [128, 8], F32, kind="ExternalInput").ap()
    gmaskT_d = nc.dram_tensor("gmask_t", [8, 128], F32, kind="ExternalInput").ap()
    if has_qkv_bias:
        qkb_d = nc.dram_tensor("qk_bias", [128, 8], F32, kind="ExternalInput").ap()
        vb_d = nc.dram_tensor("v_bias", [128, KC], F32, kind="ExternalInput").ap()
    if has_proj_bias:
        pb_d = nc.dram_tensor("p_bias", [128, KC], F32, kind="ExternalInput").ap()
    out_d = nc.dram_tensor("out", [CH, L], F32, kind="ExternalOutput").ap()
    if debug:
        dbg = {
            "d_xhat": nc.dram_tensor("d_xhat", [128, KC * L], F32, kind="ExternalOutput").ap(),
            "d_qk": nc.dram_tensor("d_qk", [128, 8 * L], F32, kind="ExternalOutput").ap(),
            "d_vt": nc.dram_tensor("d_vt", [128, SC * HEADS * 65], F32, kind="ExternalOutput").ap(),
            "d_asb": nc.dram_tensor("d_asb", [128, KC * L], F32, kind="ExternalOutput").ap(),
            "d_ew0": nc.dram_tensor("d_ew0", [128, L], F32, kind="ExternalOutput").ap(),
        }

    with tile.TileContext(nc) as tc:
        import contextlib
        ctx = contextlib.ExitStack()
        pers = ctx.enter_context(tc.tile_pool(name="pers", bufs=1))
        scr = ctx.enter_context(tc.tile_pool(name="scr", bufs=2))
        ewp = ctx.enter_context(tc.tile_pool(name="ewp", bufs=8))
        dvp = ctx.enter_context(tc.tile_pool(name="dvp", bufs=2))
        asg = ctx.enter_context(tc.tile_pool(name="asg", bufs=8))
        outp = ctx.enter_context(tc.tile_pool(name="outp", bufs=3))

        # ---- PE warmup: keep HAM at K=8/8 through the stats/DMA startup chain ----
        with tc.tile_pool(name="psW", bufs=1, space="PSUM") as psW:
            wsrc = scr.tile([128, 640], BF16, tag="wsrc")
            nc.gpsimd.memset(wsrc[:], 0.0)
            wps = psW.tile([128, 512], F32, tag="warm")
            for _ in range(20):
                nc.tensor.matmul(wps[:], wsrc[:, 0:128], wsrc[:, 128:640],
                                 start=True, stop=True)

        # ---- load inputs ----
        xs = pers.tile([128, KC * L], F32, tag="xs")
        for k in range(KC):
            nc.sync.dma_start(xs[:, k * L:(k + 1) * L], x_d[128 * k:128 * (k + 1), :])
        qw = pers.tile([128, KC * 3 * CH], BF16, tag="qw")
        for k in range(KC):
            nc.sync.dma_start(qw[:, k * 3 * CH:(k + 1) * 3 * CH],
                              qw_d[128 * k:128 * (k + 1), :])
        pw = pers.tile([128, KC * CH], BF16, tag="pw")
        for k in range(KC):
            nc.sync.dma_start(pw[:, k * CH:(k + 1) * CH], pw_d[128 * k:128 * (k + 1), :])
        gmask = pers.tile([128, 8], F32, tag="gmask")
        nc.sync.dma_start(gmask[:], gmask_d[:])
        gmaskT = pers.tile([8, 128], F32, tag="gmask_t")
        nc.sync.dma_start(gmaskT[:], gmaskT_d[:])
        if has_qkv_bias:
            qkb = pers.tile([128, 8], F32, tag="qkb")
            nc.sync.dma_start(qkb[:], qkb_d[:])
            vb = pers.tile([128, KC], F32, tag="vb")
            nc.sync.dma_start(vb[:], vb_d[:])
        if has_proj_bias:
            pb = pers.tile([128, KC], F32, tag="pb")
            nc.sync.dma_start(pb[:], pb_d[:])

        # ---- GroupNorm statistics + xhat, fully per-chunk so qkv can start on
        # chunk 0 while chunk 3 is still being reduced ----
        qkv_psum = tc.tile_pool(name="psQ", bufs=4, space="PSUM")
        psQ = qkv_psum.__enter__()
        epsb = pers.tile([8, 1], F32, tag="epsb")
        nc.gpsimd.memset(epsb[:], EPS)
        ones64 = pers.tile([1, 64], F32, tag="ones64")
        nc.gpsimd.memset(ones64[:], 1.0)
        # trigger the (single) ACT table load off the critical path
        tldt = pers.tile([8, 1], F32, tag="tldt")
        nc.scalar.activation(tldt[:], epsb[:], AF.Exp)

        stat = pers.tile([128, 8], F32, tag="stat")  # cols 2k: sum(x), 2k+1: sum(x^2)
        xhat = pers.tile([128, KC * L], BF16, tag="xhat")
        bc = pers.tile([128, 2 * KC], F32, tag="bc")  # cols 2k mean, 2k+1 rstd
        for k in range(KC):
            xk = xs[:, k * L:(k + 1) * L]
            sq = scr.tile([128, L], F32, tag="sq")
            nc.scalar.activation(sq[:], xk, AF.Identity,
                                 accum_out=stat[:, 2 * k:2 * k + 1])
            sq2 = scr.tile([128, L], F32, tag="sq")
            nc.scalar.activation(sq2[:], xk, AF.Square,
                                 accum_out=stat[:, 2 * k + 1:2 * k + 2])
            gst_ps = psQ.tile([8, 2], F32, tag="ps")
            nc.tensor.matmul(gst_ps[:], gmask[:], stat[:, 2 * k:2 * k + 2],
                             start=True, stop=True)
            s2k = pers.tile([8, 2], F32, tag=f"s2k{k}")   # col 0 mean, col 1 rstd
            vk = pers.tile([8, 2], F32, tag=f"vk{k}")     # col 0 var, col 1 scratch
            nc.vector.tensor_scalar_mul(s2k[:], gst_ps[:], 1.0 / GN_N)  # mean, E[x^2]
            nc.vector.tensor_mul(vk[:, 1:2], s2k[:, 0:1], s2k[:, 0:1])  # mean^2
            nc.vector.tensor_sub(vk[:, 0:1], s2k[:, 1:2], vk[:, 1:2])   # var
            nc.scalar.activation(vk[:, 1:2], vk[:, 0:1], AF.Ln, bias=epsb[:])
            nc.scalar.activation(s2k[:, 1:2], vk[:, 1:2], AF.Exp, scale=-0.5)
            bc_ps = psQ.tile([128, 2], F32, tag="ps")
            nc.tensor.matmul(bc_ps[:], gmaskT[:], s2k[:], start=True, stop=True)
            nc.vector.tensor_copy(bc[:, 2 * k:2 * k + 2], bc_ps[:])
            nmr = pers.tile([128, 1], F32, tag=f"nmr{k}")   # -mean*rstd
            nc.vector.tensor_scalar(
                out=nmr[:], in0=bc[:, 2 * k:2 * k + 1],
                scalar1=bc[:, 2 * k + 1:2 * k + 2], scalar2=-1.0,
                op0=OP.mult, op1=OP.mult)
            nc.scalar.activation(xhat[:, k * L:(k + 1) * L], xk, AF.Identity,
                                 bias=nmr[:], scale=bc[:, 2 * k + 1:2 * k + 2])

        if debug:
            def dump_bf16(dram_ap, sb_ap, width):
                for off in range(0, width, 512):
                    w = min(512, width - off)
                    stg = outp.tile([128, 512], F32, tag="dstg")
                    nc.vector.tensor_copy(stg[:sb_ap.shape[0], :w],
                                          sb_ap[:, off:off + w])
                    nc.sync.dma_start(dram_ap[:sb_ap.shape[0], off:off + w],
                                      stg[:sb_ap.shape[0], :w])
            dump_bf16(dbg["d_xhat"], xhat[:], KC * L)

        # ---- qkv: q,k in (o, l) layout; v transposed to (l, vc) with ones column.
        # Only pairs 0-1's q/k (j=0,4,1,5) and vT are computed up front; the
        # rest are emitted inside the attention pair loops as PE filler (the
        # attention phase is ACT-bound, in-order PE needs real work in-stream).
        qk = pers.tile([128, 8 * L], BF16, tag="qk")   # o-chunk j: cols j*L..; j=0-3 q, 4-7 k

        def emit_qk(j, pool, width):
            for t in range(TC):
                ps = pool.tile([128, width], F32, tag="ps")
                for k in range(KC):
                    nc.tensor.matmul(
                        ps[:, 0:512],
                        qw[:, k * 3 * CH + 128 * j:k * 3 * CH + 128 * (j + 1)],
                        xhat[:, k * L + 512 * t:k * L + 512 * (t + 1)],
                        start=(k == 0), stop=(k == KC - 1))
                dst = qk[:, j * L + 512 * t:j * L + 512 * (t + 1)]
                if has_qkv_bias:
                    nc.vector.tensor_scalar_add(dst, ps[:, 0:512], qkb[:, j:j + 1])
                else:
                    nc.vector.tensor_copy(dst, ps[:, 0:512])

        import contextlib as _ctxlib
        def _scope(name):
            return tc.spectator_scope(name) if SCOPES else _ctxlib.nullcontext()
        with _scope("qkv"):
            for j in ((0, 4, 1, 5) if DEFER_QK else (0, 4, 1, 5, 2, 6, 3, 7)):
                emit_qk(j, psQ, 512)
            vt = pers.tile([128, SC * (HEADS * 65)], BF16, tag="vt")
            for lc in range(SC):
                v3 = vt[:, lc * 520:(lc + 1) * 520].rearrange("p (h c) -> p h c", c=65)
                nc.gpsimd.memset(v3[:, :, 64:65], 1.0)
            for lc in range(SC):
                ps = psQ.tile([128, 512], F32, tag="ps")
                for k in range(KC):
                    nc.tensor.matmul(
                        ps[:], xhat[:, k * L + 128 * lc:k * L + 128 * (lc + 1)],
                        qw[:, k * 3 * CH + 2 * CH:k * 3 * CH + 3 * CH],
                        start=(k == 0), stop=(k == KC - 1))
                v3 = vt[:, lc * 520:(lc + 1) * 520].rearrange("p (h c) -> p h c", c=65)
                src = ps[:].rearrange("p (h c) -> p h c", c=64)
                nc.vector.tensor_copy(v3[:, :, 0:64], src)
        qkv_psum.__exit__(None, None, None)

        if debug:
            dump_bf16(dbg["d_qk"], qk[:], 8 * L)
            dump_bf16(dbg["d_vt"], vt[:], SC * HEADS * 65)

        # ---- attention, head pairs (2m, 2m+1) packed into PE row groups ----
        a_sb = pers.tile([128, KC * L], BF16, tag="a_sb")
        attn_psum = tc.tile_pool(name="psS", bufs=2, space="PSUM")
        psS = attn_psum.__enter__()
        attn_acc = tc.tile_pool(name="psA", bufs=4, space="PSUM")
        psA = attn_acc.__enter__()

        def div_recip(stgs):
            # Fold the four 512-wide ones-row sums into (128,16) via tiny
            # SBUF->SBUF DMAs (DMA engines are idle here) so the ACT ln/exp
            # for 1/den costs ~0.6us instead of ~4us of 1-partition work,
            # then unfold back to a partition-0 row for the gpsimd broadcast.
            den128 = dvp.tile([128, 16], F32, tag="d128")
            for i, (sg, e, t, mm_) in enumerate(stgs):
                nc.sync.dma_start(den128[:, 4 * i:4 * (i + 1)], sg[64:65, :])
            ln128 = dvp.tile([128, 16], F32, tag="l128")
            nc.scalar.activation(ln128[:], den128[:], AF.Ln)
            r128 = dvp.tile([128, 16], F32, tag="r128")
            nc.scalar.activation(r128[:], ln128[:], AF.Exp, scale=-1.0)
            rden = dvp.tile([1, 4 * 512], F32, tag="rden")
            for i in range(4):
                nc.sync.dma_start(rden[0:1, 512 * i:512 * (i + 1)],
                                  r128[:, 4 * i:4 * (i + 1)])
            return rden

        def div_mul(rden, i, sg, e, t, mm_):
            bsb = dvp.tile([64, 512], F32, tag="bsb")
            nc.gpsimd.partition_broadcast(bsb[:], rden[0:1, 512 * i:512 * (i + 1)])
            dst = a_sb[64 * e:64 * (e + 1),
                       mm_ * L + 512 * t:mm_ * L + 512 * (t + 1)]
            nc.vector.tensor_mul(dst, sg[0:64, :], bsb[:])
            if has_qkv_bias:
                nc.vector.tensor_scalar_add(
                    dst, dst, vb[64 * e:64 * (e + 1), mm_:mm_ + 1])

        def division_steps(stgs):
            # generator: one cheap step per scheduling slot
            rden = div_recip(stgs)
            yield
            for i, (sg, e, t, mm_) in enumerate(stgs):
                div_mul(rden, i, sg, e, t, mm_)
                if i % 2 == 1:
                    yield

        pending_div = None
        for m in range(4):
            with _scope(f"attn{m}"):
                ps_a = [[None, None], [None, None]]
                for e in range(2):
                    for t in range(TC):
                        pa = psA.tile([65, 512], F32, tag="pa")
                        ps_a[e][t] = pa

                def q_ap(e, t):
                    return qk[64 * e:64 * (e + 1), m * L + 512 * t:m * L + 512 * (t + 1)]

                def k_ap(e, sc):
                    return qk[64 * e:64 * (e + 1),
                              (4 + m) * L + 128 * sc:(4 + m) * L + 128 * (sc + 1)]

                def attn_mm(sc, e):
                    ew = ew_tiles[(sc, e)]
                    for t in range(TC):
                        nc.tensor.matmul(
                            ps_a[e][t][:],
                            vt[:, sc * 520 + (2 * m + e) * 65:
                               sc * 520 + (2 * m + e) * 65 + 65],
                            ew[:, 512 * t:512 * (t + 1)],
                            start=(sc == 0), stop=(sc == SC - 1))

                ew_tiles = {}
                for sc in range(SC):
                    ps_w = [None, None]
                    for e in range(2):
                        pw_t = psS.tile([128, 1024], F32, tag="ps")
                        ps_w[e] = pw_t
                    # packed score MM pairs (head 2m rows 0-63, head 2m+1 rows 64-127)
                    for t in range(TC):
                        for e in range(2):
                            nc.tensor.matmul(ps_w[e][:, 512 * t:512 * (t + 1)],
                                             k_ap(e, sc), q_ap(e, t),
                                             start=True, stop=True)
                    for e in range(2):
                        ew = ewp.tile([128, L], BF16, tag="ew")
                        ew_tiles[(sc, e)] = ew
                        nc.scalar.activation(ew[:], ps_w[e][:], AF.Exp)
                    if debug and m == 0 and sc == 0:
                        dump_bf16(dbg["d_ew0"], ew_tiles[(0, 0)][:], L)
                    # previous pair's division, one step per sc to spread the load
                    if pending_div is not None:
                        next(pending_div, None)
                    # deferred q/k matmuls for pair m+2 act as PE filler in the
                    # ACT-bound attention steady state
                    if DEFER_QK:
                        if m < 2 and sc == 2:
                            emit_qk(m + 2, psS, 1024)
                        if m < 2 and sc == 5:
                            emit_qk(4 + m + 2, psS, 1024)
                    # software-pipeline: attn MMs for sc-1 after scores for sc
                    if sc > 0:
                        for e in range(2):
                            attn_mm(sc - 1, e)
                for e in range(2):
                    attn_mm(SC - 1, e)

                # stage accumulators to SBUF so the PSUM banks free up for the
                # next head pair; the divisions run interleaved with the NEXT
                # pair's exp stream (pending_div) to avoid an ACT lump here.
                if pending_div is not None:
                    for _ in pending_div:  # flush any leftovers of pair m-1
                        pass
                stgs = []
                for e in range(2):
                    for t in range(TC):
                        sg = asg.tile([65, 512], F32, tag="astg")
                        nc.vector.tensor_copy(sg[:], ps_a[e][t][:])
                        stgs.append((sg, e, t, m))
                if m < 3:
                    pending_div = division_steps(stgs)
                else:
                    pending_div = None
                    final_stgs = stgs
        attn_acc.__exit__(None, None, None)
        attn_psum.__exit__(None, None, None)

        if debug:
            dump_bf16(dbg["d_asb"], a_sb[:], KC * L)

        # ---- proj + residual: k-outer waves across all 8 PSUM banks, so the
        # first 24 matmuls only need a_sb chunks 0-2 and overlap the final
        # division flush ----
        with tc.tile_pool(name="psP", bufs=6, space="PSUM") as psP, \
             tc.tile_pool(name="psB", bufs=2, space="PSUM") as psB, \
             _scope("proj"):
            for t in range(TC):
                pstiles = {}
                for i in range(KC):
                    ps = psP.tile([128, 512], F32, tag="ps")
                    pstiles[i] = ps
                for k in range(KC):
                    if t == 0 and k == 1:
                        # final pair's reciprocal: DMA-fold + ACT, off the PE stream
                        final_rden = div_recip(final_stgs)
                    if t == 0 and k == 3:
                        # broadcast on the (briefly idle) PE into spare banks
                        for i_, (sg, e, tt, mm_) in enumerate(final_stgs):
                            pb_ps = psB.tile([64, 512], F32, tag="pb")
                            nc.tensor.matmul(
                                pb_ps[:], ones64[:],
                                final_rden[0:1, 512 * i_:512 * (i_ + 1)],
                                start=True, stop=True)
                            dst = a_sb[64 * e:64 * (e + 1),
                                       mm_ * L + 512 * tt:mm_ * L + 512 * (tt + 1)]
                            nc.vector.tensor_mul(dst, sg[0:64, :], pb_ps[:])
                            if has_qkv_bias:
                                nc.vector.tensor_scalar_add(
                                    dst, dst, vb[64 * e:64 * (e + 1), mm_:mm_ + 1])
                    for i in range(KC):
                        nc.tensor.matmul(
                            pstiles[i][:],
                            pw[:, k * CH + 128 * i:k * CH + 128 * (i + 1)],
                            a_sb[:, k * L + 512 * t:k * L + 512 * (t + 1)],
                            start=(k == 0), stop=(k == KC - 1))
                for i in range(KC):
                    ot = outp.tile([128, 512], F32, tag="ot")
                    nc.vector.tensor_add(ot[:],
                                         xs[:, i * L + 512 * t:i * L + 512 * (t + 1)],
                                         pstiles[i][:])
                    if has_proj_bias:
                        nc.vector.tensor_scalar_add(ot[:], ot[:], pb[:, i:i + 1])
                    nc.sync.dma_start(
                        out_d[128 * i:128 * (i + 1), 512 * t:512 * (t + 1)], ot[:])
        ctx.close()

    nc.compile()
    return nc


def _prep_inputs(x, norm_w, norm_b, qkv_w, qkv_b, proj_w, proj_b):
    scale = DH ** -0.25
    w_eff = (qkv_w.astype(np.float64) * norm_w.astype(np.float64)[None, :])
    b_eff = qkv_b.astype(np.float64) + w_eff @ norm_b.astype(np.float64)
    # reference splits qkv per head: row h*192 + {0:64 q, 64:128 k, 128:192 v}.
    # device layout wants [q_all_heads | k_all_heads | v_all_heads], head-major.
    perm = np.concatenate([
        np.concatenate([np.arange(h * 3 * DH + t * DH, h * 3 * DH + (t + 1) * DH)
                        for h in range(HEADS)])
        for t in range(3)])
    w_eff = w_eff[perm]
    b_eff = b_eff[perm]
    w_eff[:2 * CH] *= scale
    b_eff[:2 * CH] *= scale
    qkv_wt = np.ascontiguousarray(w_eff.T).astype(np.float32).astype(ml_dtypes.bfloat16)
    proj_wt = np.ascontiguousarray(proj_w.T).astype(ml_dtypes.bfloat16)

    p = np.arange(128)
    gmask = (p[:, None] // 16 == np.arange(8)[None, :]).astype(np.float32)
    gmask_t = np.ascontiguousarray(gmask.T)

    has_qkv_bias = bool(np.any(b_eff != 0.0))
    has_proj_bias = bool(np.any(proj_b != 0.0))
    common = {"qkv_wt": qkv_wt, "proj_wt": proj_wt, "gmask": gmask,
              "gmask_t": gmask_t}
    if has_qkv_bias:
        qk_part = b_eff[:2 * CH].astype(np.float32).reshape(8, 128).T
        v_part = b_eff[2 * CH:].astype(np.float32).reshape(KC, 128).T
        common["qk_bias"] = np.ascontiguousarray(qk_part)
        common["v_bias"] = np.ascontiguousarray(v_part)
    if has_proj_bias:
        common["p_bias"] = np.ascontiguousarray(
            proj_b.astype(np.float32).reshape(KC, 128).T)
    xf = np.ascontiguousarray(x.reshape(B, CH, L)).astype(np.float32)
    in_maps = [dict(common, x=np.ascontiguousarray(xf[i])) for i in range(B)]
    return in_maps, has_qkv_bias, has_proj_bias


def _get_nc(flags):
    if flags not in _cache:
        _cache[flags] = _build(*flags)
    return _cache[flags]


def _run(inputs, trace=False, tmpdir=None):
    import time
    from concourse.bass_utils import run_bass_kernel_spmd
    in_maps, hqb, hpb = _prep_inputs(**inputs)
    nc = _get_nc((hqb, hpb))
    kw = {}
    if trace:
        kw = dict(trace=True, tmpdir=tmpdir)
    last_err = None
    for attempt in range(3):
        # the very first execution on a freshly-attached device occasionally
        # fails with NRT_EXEC_UNIT_UNRECOVERABLE; a retry recovers it
        try:
            res = run_bass_kernel_spmd(nc, in_maps, list(range(B)), **kw)
            break
        except Exception as e:  # noqa: BLE001
            last_err = e
            time.sleep(5)
    else:
        raise last_err
    out = np.stack([res.results[i]["out"] for i in range(B)])
    return out.reshape(B, CH, HH, WW).astype(np.float32), res


def kernel(x, norm_w, norm_b, qkv_w, qkv_b, proj_w, proj_b):
    out, _ = _run(dict(x=x, norm_w=norm_w, norm_b=norm_b, qkv_w=qkv_w,
                       qkv_b=qkv_b, proj_w=proj_w, proj_b=proj_b))
    return out


# revision 68
# speedup vs baseline: 1.0590x; 1.0269x over previous
"""TRN2 Bass kernel for nn_AttentionBlock (GroupNorm32 + 8-head attention + proj + residual).

Sharding: data-parallel over batch — batch=8, one batch element per NeuronCore, no
collectives.

Per core: GroupNorm stats per 128-channel chunk (sum and sum-of-squares fall out of
ACT Identity/Square activations via accum_out, grouped by tiny mask matmuls,
rsqrt as exp(-0.5*ln v) so one ACT table set serves the whole kernel); qkv, attention
and proj as bf16 matmuls on TensorE (score matmuls for a head pair run concurrently
in the two 64-row PE groups); softmax exp on ScalarE in one (128,1024) activation per
score block; the attention matmul uses vT with an appended ones-column so the softmax
denominator falls out of the same accumulation, and the division is 1/den =
exp(-ln den): the four 512-wide denominator rows of a head pair are folded to
(128,16) by tiny SBUF->SBUF DMAs so the ACT ln/exp is ~0.6us, unfolded back,
broadcast on GpSimd (final pair: tiny f32 PE matmuls into spare
PSUM banks, since the PE is briefly idle there) and multiplied on DVE.
Schedule-shaping for the in-order engines: PE warm-up matmuls cover the stats
startup, later head-pairs' q/k matmuls are deferred into earlier pairs' loops as PE
filler (the attention steady state is ACT-bound), divisions are software-pipelined
into the next pair's loop, and proj runs k-outer waves across all 8 PSUM banks.

Numerics: all matmuls bf16 with fp32 PSUM accumulation (end-to-end ~1.9e-4 rel-l2 vs
the fp32 reference); everything else fp32.

Self-contained: hardcodes shapes from the problem spec (x (8,512,32,32) f32 etc).
"""
import numpy as np
import ml_dtypes

B, CH, HH, WW = 8, 512, 32, 32
L = HH * WW                  # 1024
HEADS = 8
GROUPS = 32
EPS = 1e-5
DH = CH // HEADS             # 64
KC = CH // 128               # 4 c-chunks
OC3 = 3 * CH // 128          # 12 qkv o-chunks
SC = L // 128                # 8 s/l-chunks
TC = L // 512                # 2 t-chunks
GN_N = (CH // GROUPS) * L    # elements per group = 16384
DEFER_QK = True
SCOPES = False

_cache = {}


def _build(has_qkv_bias, has_proj_bias, debug=False):
    import concourse.bass as bass
    import concourse.tile as tile
    from concourse import bacc, mybir
    import bass_rust as _bass_rust
    from concourse.hw_specs import get_activation_tables

    F32 = mybir.dt.float32
    BF16 = mybir.dt.bfloat16
    AF = mybir.ActivationFunctionType
    OP = mybir.AluOpType
    AX = mybir.AxisListType

    class _Bacc(bacc.Bacc):
        # Pin Exp/Ln to the combined `natural_log_exp_and_others` table set so
        # alternating Ln/Exp activations don't thrash ACT_TABLE_LOADs (~2.7us
        # each). Same algorithm as Bacc.insert_act_table_loads, with Exp/Ln
        # stripped from every other set so the chooser can't pick them.
        def insert_act_table_loads(self):
            has_activation = any(
                isinstance(i, mybir.InstActivation)
                for b in self.main_func.blocks
                for i in b.instructions
            )
            if not has_activation:
                return
            combo = {AF.Exp, AF.Ln}
            tables = []
            for name, fns in get_activation_tables(self.m.arch).items():
                if name != "natural_log_exp_and_others":
                    fns = {f for f in fns if f not in combo}
                tables.append((name, fns))
            _bass_rust.insert_act_table_loads(self, tables)

    nc = _Bacc("TRN2", target_bir_lowering=False, debug=False, num_devices=8)

    x_d = nc.dram_tensor("x", [CH, L], F32, kind="ExternalInput").ap()
    qw_d = nc.dram_tensor("qkv_wt", [CH, 3 * CH], BF16, kind="ExternalInput").ap()
    pw_d = nc.dram_tensor("proj_wt", [CH, CH], BF16, kind="ExternalInput").ap()
    gmask_d = nc.dram_tensor("gmask", [128, 8], F32, kind="ExternalInput").ap()
    gmaskT_d = nc.dram_tensor("gmask_t", [8, 128], F32, kind="ExternalInput").ap()
    if has_qkv_bias:
        qkb_d = nc.dram_tensor("qk_bias", [128, 8], F32, kind="ExternalInput").ap()
        vb_d = nc.dram_tensor("v_bias", [128, KC], F32, kind="ExternalInput").ap()
    if has_proj_bias:
        pb_d = nc.dram_tensor("p_bias", [128, KC], F32, kind="ExternalInput").ap()
    out_d = nc.dram_tensor("out", [CH, L], F32, kind="ExternalOutput").ap()
    if debug:
        dbg = {
            "d_xhat": nc.dram_tensor("d_xhat", [128, KC * L], F32, kind="ExternalOutput").ap(),
            "d_qk": nc.dram_tensor("d_qk", [128, 8 * L], F32, kind="ExternalOutput").ap(),
            "d_vt": nc.dram_tensor("d_vt", [128, SC * HEADS * 65], F32, kind="ExternalOutput").ap(),
            "d_asb": nc.dram_tensor("d_asb", [128, KC * L], F32, kind="ExternalOutput").ap(),
            "d_ew0": nc.dram_tensor("d_ew0", [128, L], F32, kind="ExternalOutput").ap(),
        }

    with tile.TileContext(nc) as tc:
        import contextlib
        ctx = contextlib.ExitStack()
        pers = ctx.enter_context(tc.tile_pool(name="pers", bufs=1))
        scr = ctx.enter_context(tc.tile_pool(name="scr", bufs=2))
        ewp = ctx.enter_context(tc.tile_pool(name="ewp", bufs=8))
        dvp = ctx.enter_context(tc.tile_pool(name="dvp", bufs=2))
        asg = ctx.enter_context(tc.tile_pool(name="asg", bufs=8))
        outp = ctx.enter_context(tc.tile_pool(name="outp", bufs=3))

        # ---- PE warmup: keep HAM at K=8/8 through the stats/DMA startup chain ----
        with tc.tile_pool(name="psW", bufs=1, space="PSUM") as psW:
            wsrc = scr.tile([128, 640], BF16, tag="wsrc")
            nc.gpsimd.memset(wsrc[:], 0.0)
            wps = psW.tile([128, 512], F32, tag="warm")
            for _ in range(64):
                nc.tensor.matmul(wps[:], wsrc[:, 0:128], wsrc[:, 128:640],
                                 start=True, stop=True)

        # ---- load inputs ----
        xs = pers.tile([128, KC * L], F32, tag="xs")
        for k in range(KC):
            nc.sync.dma_start(xs[:, k * L:(k + 1) * L], x_d[128 * k:128 * (k + 1), :])
        qw = pers.tile([128, KC * 3 * CH], BF16, tag="qw")
        for k in range(KC):
            nc.sync.dma_start(qw[:, k * 3 * CH:(k + 1) * 3 * CH],
                              qw_d[128 * k:128 * (k + 1), :])
        pw = pers.tile([128, KC * CH], BF16, tag="pw")
        for k in range(KC):
            nc.sync.dma_start(pw[:, k * CH:(k + 1) * CH], pw_d[128 * k:128 * (k + 1), :])
        gmask = pers.tile([128, 8], F32, tag="gmask")
        nc.sync.dma_start(gmask[:], gmask_d[:])
        gmaskT = pers.tile([8, 128], F32, tag="gmask_t")
        nc.sync.dma_start(gmaskT[:], gmaskT_d[:])
        if has_qkv_bias:
            qkb = pers.tile([128, 8], F32, tag="qkb")
            nc.sync.dma_start(qkb[:], qkb_d[:])
            vb = pers.tile([128, KC], F32, tag="vb")
            nc.sync.dma_start(vb[:], vb_d[:])
        if has_proj_bias:
            pb = pers.tile([128, KC], F32, tag="pb")
            nc.sync.dma_start(pb[:], pb_d[:])

        # ---- GroupNorm statistics + xhat, fully per-chunk so qkv can start on
        # chunk 0 while chunk 3 is still being reduced ----
        qkv_psum = tc.tile_pool(name="psQ", bufs=4, space="PSUM")
        psQ = qkv_psum.__enter__()
        epsb = pers.tile([8, 1], F32, tag="epsb")
        nc.gpsimd.memset(epsb[:], EPS)
        ones64 = pers.tile([1, 64], F32, tag="ones64")
        nc.gpsimd.memset(ones64[:], 1.0)
        # trigger the (single) ACT table load off the critical path
        tldt = pers.tile([8, 1], F32, tag="tldt")
        nc.scalar.activation(tldt[:], epsb[:], AF.Exp)

        stat = pers.tile([128, 8], F32, tag="stat")  # cols 2k: sum(x), 2k+1: sum(x^2)
        xhat = pers.tile([128, KC * L], BF16, tag="xhat")
        bc = pers.tile([128, 2 * KC], F32, tag="bc")  # cols 2k mean, 2k+1 rstd
        for k in range(KC):
            xk = xs[:, k * L:(k + 1) * L]
            sq = scr.tile([128, L], F32, tag="sq")
            nc.scalar.activation(sq[:], xk, AF.Identity,
                                 accum_out=stat[:, 2 * k:2 * k + 1])
            sq2 = scr.tile([128, L], F32, tag="sq")
            nc.scalar.activation(sq2[:], xk, AF.Square,
                                 accum_out=stat[:, 2 * k + 1:2 * k + 2])
            gst_ps = psQ.tile([8, 2], F32, tag="ps")
            nc.tensor.matmul(gst_ps[:], gmask[:], stat[:, 2 * k:2 * k + 2],
                             start=True, stop=True)
            s2k = pers.tile([8, 2], F32, tag=f"s2k{k}")   # col 0 mean, col 1 rstd
            vk = pers.tile([8, 2], F32, tag=f"vk{k}")     # col 0 var, col 1 scratch
            nc.vector.tensor_scalar_mul(s2k[:], gst_ps[:], 1.0 / GN_N)  # mean, E[x^2]
            nc.vector.tensor_mul(vk[:, 1:2], s2k[:, 0:1], s2k[:, 0:1])  # mean^2
            nc.vector.tensor_sub(vk[:, 0:1], s2k[:, 1:2], vk[:, 1:2])   # var
            nc.scalar.activation(vk[:, 1:2], vk[:, 0:1], AF.Ln, bias=epsb[:])
            nc.scalar.activation(s2k[:, 1:2], vk[:, 1:2], AF.Exp, scale=-0.5)
            bc_ps = psQ.tile([128, 2], F32, tag="ps")
            nc.tensor.matmul(bc_ps[:], gmaskT[:], s2k[:], start=True, stop=True)
            nc.vector.tensor_copy(bc[:, 2 * k:2 * k + 2], bc_ps[:])
            nmr = pers.tile([128, 1], F32, tag=f"nmr{k}")   # -mean*rstd
            nc.vector.tensor_scalar(
                out=nmr[:], in0=bc[:, 2 * k:2 * k + 1],
                scalar1=bc[:, 2 * k + 1:2 * k + 2], scalar2=-1.0,
                op0=OP.mult, op1=OP.mult)
            nc.scalar.activation(xhat[:, k * L:(k + 1) * L], xk, AF.Identity,
                                 bias=nmr[:], scale=bc[:, 2 * k + 1:2 * k + 2])

        if debug:
            def dump_bf16(dram_ap, sb_ap, width):
                for off in range(0, width, 512):
                    w = min(512, width - off)
                    stg = outp.tile([128, 512], F32, tag="dstg")
                    nc.vector.tensor_copy(stg[:sb_ap.shape[0], :w],
                                          sb_ap[:, off:off + w])
                    nc.sync.dma_start(dram_ap[:sb_ap.shape[0], off:off + w],
                                      stg[:sb_ap.shape[0], :w])
            dump_bf16(dbg["d_xhat"], xhat[:], KC * L)

        # ---- qkv: q,k in (o, l) layout; v transposed to (l, vc) with ones column.
        # Only pairs 0-1's q/k (j=0,4,1,5) and vT are computed up front; the
        # rest are emitted inside the attention pair loops as PE filler (the
        # attention phase is ACT-bound, in-order PE needs real work in-stream).
        qk = pers.tile([128, 8 * L], BF16, tag="qk")   # o-chunk j: cols j*L..; j=0-3 q, 4-7 k

        def emit_qk(j, pool, width):
            for t in range(TC):
                ps = pool.tile([128, width], F32, tag="ps")
                for k in range(KC):
                    nc.tensor.matmul(
                        ps[:, 0:512],
                        qw[:, k * 3 * CH + 128 * j:k * 3 * CH + 128 * (j + 1)],
                        xhat[:, k * L + 512 * t:k * L + 512 * (t + 1)],
                        start=(k == 0), stop=(k == KC - 1))
                dst = qk[:, j * L + 512 * t:j * L + 512 * (t + 1)]
                if has_qkv_bias:
                    nc.vector.tensor_scalar_add(dst, ps[:, 0:512], qkb[:, j:j + 1])
                else:
                    nc.vector.tensor_copy(dst, ps[:, 0:512])

        import contextlib as _ctxlib
        def _scope(name):
            return tc.spectator_scope(name) if SCOPES else _ctxlib.nullcontext()
        with _scope("qkv"):
            for j in ((0, 4, 1, 5) if DEFER_QK else (0, 4, 1, 5, 2, 6, 3, 7)):
                emit_qk(j, psQ, 512)
            vt = pers.tile([128, SC * (HEADS * 65)], BF16, tag="vt")
            for lc in range(SC):
                v3 = vt[:, lc * 520:(lc + 1) * 520].rearrange("p (h c) -> p h c", c=65)
                nc.gpsimd.memset(v3[:, :, 64:65], 1.0)
            for lc in range(SC):
                ps = psQ.tile([128, 512], F32, tag="ps")
                for k in range(KC):
                    nc.tensor.matmul(
                        ps[:], xhat[:, k * L + 128 * lc:k * L + 128 * (lc + 1)],
                        qw[:, k * 3 * CH + 2 * CH:k * 3 * CH + 3 * CH],
                        start=(k == 0), stop=(k == KC - 1))
                v3 = vt[:, lc * 520:(lc + 1) * 520].rearrange("p (h c) -> p h c", c=65)
                src = ps[:].rearrange("p (h c) -> p h c", c=64)
                nc.vector.tensor_copy(v3[:, :, 0:64], src)
        qkv_psum.__exit__(None, None, None)

        if debug:
            dump_bf16(dbg["d_qk"], qk[:], 8 * L)
            dump_bf16(dbg["d_vt"], vt[:], SC * HEADS * 65)

        # ---- attention, head pairs (2m, 2m+1) packed into PE row groups ----
        a_sb = pers.tile([128, KC * L], BF16, tag="a_sb")
        attn_psum = tc.tile_pool(name="psS", bufs=2, space="PSUM")
        psS = attn_psum.__enter__()
        attn_acc = tc.tile_pool(name="psA", bufs=4, space="PSUM")
        psA = attn_acc.__enter__()

        def div_recip(stgs):
            # Fold the four 512-wide ones-row sums into (128,16) via tiny
            # SBUF->SBUF DMAs (DMA engines are idle here) so the ACT ln/exp
            # for 1/den costs ~0.6us instead of ~4us of 1-partition work,
            # then unfold back to a partition-0 row for the gpsimd broadcast.
            den128 = dvp.tile([128, 16], F32, tag="d128")
            for i, (sg, e, t, mm_) in enumerate(stgs):
                nc.sync.dma_start(den128[:, 4 * i:4 * (i + 1)], sg[64:65, :])
            ln128 = dvp.tile([128, 16], F32, tag="l128")
            nc.scalar.activation(ln128[:], den128[:], AF.Ln)
            r128 = dvp.tile([128, 16], F32, tag="r128")
            nc.scalar.activation(r128[:], ln128[:], AF.Exp, scale=-1.0)
            rden = dvp.tile([1, 4 * 512], F32, tag="rden")
            for i in range(4):
                nc.sync.dma_start(rden[0:1, 512 * i:512 * (i + 1)],
                                  r128[:, 4 * i:4 * (i + 1)])
            return rden

        def div_mul(rden, i, sg, e, t, mm_):
            bsb = dvp.tile([64, 512], F32, tag="bsb")
            nc.gpsimd.partition_broadcast(bsb[:], rden[0:1, 512 * i:512 * (i + 1)])
            dst = a_sb[64 * e:64 * (e + 1),
                       mm_ * L + 512 * t:mm_ * L + 512 * (t + 1)]
            nc.vector.tensor_mul(dst, sg[0:64, :], bsb[:])
            if has_qkv_bias:
                nc.vector.tensor_scalar_add(
                    dst, dst, vb[64 * e:64 * (e + 1), mm_:mm_ + 1])

        def division_steps(stgs):
            # generator: one cheap step per scheduling slot
            rden = div_recip(stgs)
            yield
            for i, (sg, e, t, mm_) in enumerate(stgs):
                div_mul(rden, i, sg, e, t, mm_)
                if i % 2 == 1:
                    yield

        pending_div = None
        for m in range(4):
            with _scope(f"attn{m}"):
                ps_a = [[None, None], [None, None]]
                for e in range(2):
                    for t in range(TC):
                        pa = psA.tile([65, 512], F32, tag="pa")
                        ps_a[e][t] = pa

                def q_ap(e, t):
                    return qk[64 * e:64 * (e + 1), m * L + 512 * t:m * L + 512 * (t + 1)]

                def k_ap(e, sc):
                    return qk[64 * e:64 * (e + 1),
                              (4 + m) * L + 128 * sc:(4 + m) * L + 128 * (sc + 1)]

                def attn_mm(sc, e):
                    ew = ew_tiles[(sc, e)]
                    for t in range(TC):
                        nc.tensor.matmul(
                            ps_a[e][t][:],
                            vt[:, sc * 520 + (2 * m + e) * 65:
                               sc * 520 + (2 * m + e) * 65 + 65],
                            ew[:, 512 * t:512 * (t + 1)],
                            start=(sc == 0), stop=(sc == SC - 1))

                ew_tiles = {}
                for sc in range(SC):
                    ps_w = [None, None]
                    for e in range(2):
                        pw_t = psS.tile([128, 1024], F32, tag="ps")
                        ps_w[e] = pw_t
                    # packed score MM pairs (head 2m rows 0-63, head 2m+1 rows 64-127)
                    for t in range(TC):
                        for e in range(2):
                            nc.tensor.matmul(ps_w[e][:, 512 * t:512 * (t + 1)],
                                             k_ap(e, sc), q_ap(e, t),
                                             start=True, stop=True)
                    for e in range(2):
                        ew = ewp.tile([128, L], BF16, tag="ew")
                        ew_tiles[(sc, e)] = ew
                        nc.scalar.activation(ew[:], ps_w[e][:], AF.Exp)
                    if debug and m == 0 and sc == 0:
                        dump_bf16(dbg["d_ew0"], ew_tiles[(0, 0)][:], L)
                    # previous pair's division, one step per sc to spread the load
                    if pending_div is not None:
                        next(pending_div, None)
                    # deferred q/k matmuls for pair m+2 act as PE filler in the
                    # ACT-bound attention steady state
                    if DEFER_QK:
                        if m < 2 and sc == 2:
                            emit_qk(m + 2, psS, 1024)
                        if m < 2 and sc == 5:
                            emit_qk(4 + m + 2, psS, 1024)
                    # software-pipeline: attn MMs for sc-1 after scores for sc
                    if sc > 0:
                        for e in range(2):
                            attn_mm(sc - 1, e)
                for e in range(2):
                    attn_mm(SC - 1, e)

                # stage accumulators to SBUF so the PSUM banks free up for the
                # next head pair; the divisions run interleaved with the NEXT
                # pair's exp stream (pending_div) to avoid an ACT lump here.
                if pending_div is not None:
                    for _ in pending_div:  # flush any leftovers of pair m-1
                        pass
                stgs = []
                for e in range(2):
                    for t in range(TC):
                        sg = asg.tile([65, 512], F32, tag="astg")
                        nc.vector.tensor_copy(sg[:], ps_a[e][t][:])
                        stgs.append((sg, e, t, m))
                if m < 3:
                    pending_div = division_steps(stgs)
                else:
                    pending_div = None
                    final_stgs = stgs
        attn_acc.__exit__(None, None, None)
        attn_psum.__exit__(None, None, None)

        if debug:
            dump_bf16(dbg["d_asb"], a_sb[:], KC * L)

        # ---- proj + residual: k-outer waves across all 8 PSUM banks, so the
        # first 24 matmuls only need a_sb chunks 0-2 and overlap the final
        # division flush ----
        with tc.tile_pool(name="psP", bufs=6, space="PSUM") as psP, \
             tc.tile_pool(name="psB", bufs=2, space="PSUM") as psB, \
             _scope("proj"):
            for t in range(TC):
                pstiles = {}
                for i in range(KC):
                    ps = psP.tile([128, 512], F32, tag="ps")
                    pstiles[i] = ps
                for k in range(KC):
                    if t == 0 and k == 1:
                        # final pair's reciprocal: DMA-fold + ACT, off the PE stream
                        final_rden = div_recip(final_stgs)
                    if t == 0 and k == 3:
                        # broadcast on the (briefly idle) PE into spare banks
                        for i_, (sg, e, tt, mm_) in enumerate(final_stgs):
                            pb_ps = psB.tile([64, 512], F32, tag="pb")
                            nc.tensor.matmul(
                                pb_ps[:], ones64[:],
                                final_rden[0:1, 512 * i_:512 * (i_ + 1)],
                                start=True, stop=True)
                            dst = a_sb[64 * e:64 * (e + 1),
                                       mm_ * L + 512 * tt:mm_ * L + 512 * (tt + 1)]
                            nc.vector.tensor_mul(dst, sg[0:64, :], pb_ps[:])
                            if has_qkv_bias:
                                nc.vector.tensor_scalar_add(
                                    dst, dst, vb[64 * e:64 * (e + 1), mm_:mm_ + 1])
                    for i in range(KC):
                        nc.tensor.matmul(
                            pstiles[i][:],
                            pw[:, k * CH + 128 * i:k * CH + 128 * (i + 1)],
                            a_sb[:, k * L + 512 * t:k * L + 512 * (t + 1)],
                            start=(k == 0), stop=(k == KC - 1))
                for i in range(KC):
                    ot = outp.tile([128, 512], F32, tag="ot")
                    nc.vector.tensor_add(ot[:],
                                         xs[:, i * L + 512 * t:i * L + 512 * (t + 1)],
                                         pstiles[i][:])
                    if has_proj_bias:
                        nc.vector.tensor_scalar_add(ot[:], ot[:], pb[:, i:i + 1])
                    nc.sync.dma_start(
                        out_d[128 * i:128 * (i + 1), 512 * t:512 * (t + 1)], ot[:])
        ctx.close()

    nc.compile()
    return nc


def _prep_inputs(x, norm_w, norm_b, qkv_w, qkv_b, proj_w, proj_b):
    scale = DH ** -0.25
    w_eff = (qkv_w.astype(np.float64) * norm_w.astype(np.float64)[None, :])
    b_eff = qkv_b.astype(np.float64) + w_eff @ norm_b.astype(np.float64)
    # reference splits qkv per head: row h*192 + {0:64 q, 64:128 k, 128:192 v}.
    # device layout wants [q_all_heads | k_all_heads | v_all_heads], head-major.
    perm = np.concatenate([
        np.concatenate([np.arange(h * 3 * DH + t * DH, h * 3 * DH + (t + 1) * DH)
                        for h in range(HEADS)])
        for t in range(3)])
    w_eff = w_eff[perm]
    b_eff = b_eff[perm]
    w_eff[:2 * CH] *= scale
    b_eff[:2 * CH] *= scale
    qkv_wt = np.ascontiguousarray(w_eff.T).astype(np.float32).astype(ml_dtypes.bfloat16)
    proj_wt = np.ascontiguousarray(proj_w.T).astype(ml_dtypes.bfloat16)

    p = np.arange(128)
    gmask = (p[:, None] // 16 == np.arange(8)[None, :]).astype(np.float32)
    gmask_t = np.ascontiguousarray(gmask.T)

    has_qkv_bias = bool(np.any(b_eff != 0.0))
    has_proj_bias = bool(np.any(proj_b != 0.0))
    common = {"qkv_wt": qkv_wt, "proj_wt": proj_wt, "gmask": gmask,
              "gmask_t": gmask_t}
    if has_qkv_bias:
        qk_part = b_eff[:2 * CH].astype(np.float32).reshape(8, 128).T
        v_part = b_eff[2 * CH:].astype(np.float32).reshape(KC, 128).T
        common["qk_bias"] = np.ascontiguousarray(qk_part)
        common["v_bias"] = np.ascontiguousarray(v_part)
    if has_proj_bias:
        common["p_bias"] = np.ascontiguousarray(
            proj_b.astype(np.float32).reshape(KC, 128).T)
    xf = np.ascontiguousarray(x.reshape(B, CH, L)).astype(np.float32)
    in_maps = [dict(common, x=np.ascontiguousarray(xf[i])) for i in range(B)]
    return in_maps, has_qkv_bias, has_proj_bias


def _get_nc(flags):
    if flags not in _cache:
        _cache[flags] = _build(*flags)
    return _cache[flags]


def _run(inputs, trace=False, tmpdir=None):
    import time
    from concourse.bass_utils import run_bass_kernel_spmd
    in_maps, hqb, hpb = _prep_inputs(**inputs)
    nc = _get_nc((hqb, hpb))
    kw = {}
    if trace:
        kw = dict(trace=True, tmpdir=tmpdir)
    last_err = None
    for attempt in range(3):
        # the very first execution on a freshly-attached device occasionally
        # fails with NRT_EXEC_UNIT_UNRECOVERABLE; a retry recovers it
        try:
            res = run_bass_kernel_spmd(nc, in_maps, list(range(B)), **kw)
            break
        except Exception as e:  # noqa: BLE001
            last_err = e
            time.sleep(5)
    else:
        raise last_err
    out = np.stack([res.results[i]["out"] for i in range(B)])
    return out.reshape(B, CH, HH, WW).astype(np.float32), res


def kernel(x, norm_w, norm_b, qkv_w, qkv_b, proj_w, proj_b):
    out, _ = _run(dict(x=x, norm_w=norm_w, norm_b=norm_b, qkv_w=qkv_w,
                       qkv_b=qkv_b, proj_w=proj_w, proj_b=proj_b))
    return out
